# revision 15
# baseline (speedup 1.0000x reference)
"""Trainium2 Bass kernel for BasicQuadRGBModel (quad-Bayer demosaic CNN).

Data-parallel over 8 cores (2 images each). bf16 datapath, f32 PSUM.

Design (v7):
  - conv = 3 accumulating matmuls/layer (K=120, M=96 padded to 128, N=512 =
    8 rows x 64 win). Stationaries zero-padded to 128 columns so the PE's
    fast-weight-load path (FWL, 128-col requirement) engages.
  - Row-slab steps of 8 rows grouped 4-at-a-time into [*, 34, 64] SBUF tiles
    (32 rows + 1 halo row each side): halo copies, edge memsets, strip DMAs
    and HBM in/out DMAs amortize 4x. f/w tree pairs share one
    [120, 2, 34, 64] tile per layer so one strip DMA serves both trees.
  - Strip DMAs split rows 0:33 (issued by the producing stage) from row 33
    (tiny DMA in the consuming stage) so layer N+1's first convs never wait
    on the next group's halo; r0 input is prefetched one group ahead.
  - 5-deep software pipeline (L0 / L1 / L2+exp / softmax-combine / chroma);
    the softmax-combine matmuls run one group behind the exp stage so the
    PE in-order stream never waits on Act/DVE results of the same step.
  - Softmax: E = exp(relu(w)) via Act relu+exp; EP = relu(f)*E fused on DVE;
    the sum matmul duplicates both half-slots (M=16) so
    reciprocal_approx_fast applies directly (no broadcast matmul); g = EP
    sums * 1/E sums on DVE.
  - g lives at partition base 64 so d = rb - g is a single fused DVE op
    against rt[64:84] (equal-SBUF-base rule for 2-input ops).
  - Engine placement: Act = L0w/L1w evictions + relu/exp + chroma staging;
    DVE = L0f/L1f evictions + fused softmax/chroma ops; GpSimd = halo
    copies, memsets, bulk strip DMA issue; Sync = HBM in/out + small strips.
  - Host does layer-0 im2col (r0) and the final 2x2 pixel shuffle.
"""

import sys

sys.path.insert(0, "/opt/trn_rl_repo")

import ml_dtypes
import numpy as np

import concourse.bass as bass
import concourse.mybir as mybir
import concourse.tile as tile
from concourse import bacc
from concourse.bass_utils import run_bass_kernel_spmd

N_CORES = 8
B_PC = 2
H = W = 512
NW = 64
NSLAB = 64
GROUP = 4
RG = 8 * GROUP  # rows per group
GPI = NSLAB // GROUP  # groups per image
NG = B_PC * GPI
CH = 12
F32 = mybir.dt.float32
BF = mybir.dt.bfloat16
NPBF = ml_dtypes.bfloat16


def _rbloc(xa, c):
    if xa == 0:
        return 16 + c
    if xa == 9:
        return 18 + c
    return (xa - 1) * 2 + c


def _rloc(xa, ci):
    if xa == 0:
        return 96 + ci
    if xa == 9:
        return 108 + ci
    return (xa - 1) * 12 + ci


def _r0loc(ky, ci, xa):
    if ky == 0:
        if ci == 0:
            return xa
        if ci == 3:
            return 10 + xa
        return 20 + _rbloc(xa, ci - 1)
    if ky == 1:
        if ci == 0:
            return 40 + xa
        if ci == 3:
            return 50 + xa
        return 64 + _rbloc(xa, ci - 1)
    if ci == 0:
        return 84 + xa
    if ci == 3:
        return 94 + xa
    return 104 + _rbloc(xa, ci - 1)


def build_r0(mosaic):
    B = mosaic.shape[0]
    mp = np.zeros((B, 4, H + 2, W + 2), np.float32)
    mp[:, :, 1 : H + 1, 1 : W + 1] = mosaic
    r0 = np.zeros((B, 128, H, NW), np.float32)
    for ky in range(3):
        for ci in range(4):
            for xa in range(10):
                r0[:, _r0loc(ky, ci, xa)] = mp[:, ci, ky : ky + H, xa : xa + 8 * NW : 8]
    return r0.astype(NPBF)


def build_w_l0(wt):
    W_ = np.zeros((128, 96), np.float32)
    for ky in range(3):
        for ci in range(4):
            for xa in range(10):
                for xo in range(8):
                    kx = xa - xo
                    if 0 <= kx <= 2:
                        for co in range(CH):
                            W_[_r0loc(ky, ci, xa), xo * 12 + co] = wt[co, ci, ky, kx]
    return W_


def build_w_int(wt):
    W_ = np.zeros((3, 120, 96), np.float32)
    for ky in range(3):
        for xa in range(10):
            for xo in range(8):
                kx = xa - xo
                if 0 <= kx <= 2:
                    k = _rloc(xa, 0)
                    W_[ky, k : k + 12, xo * 12 : xo * 12 + 12] = wt[:, :, ky, kx].T
    return W_


def build_w_sums():
    # wse2 duplicates the full per-xo sum into both half slots so the
    # reciprocal applies directly to the 16-partition layout (no broadcast
    # matmul needed).
    wse2 = np.zeros((96, 16), np.float32)
    wsep = np.zeros((96, 16), np.float32)
    for xo in range(8):
        for co in range(CH):
            wse2[xo * 12 + co, xo * 2] = 1.0
            wse2[xo * 12 + co, xo * 2 + 1] = 1.0
            wsep[xo * 12 + co, xo * 2 + (co >= 6)] = 1.0
    return wse2, wsep


def build_w_chroma(cw0):
    wchk = np.zeros((3, 20, 48), np.float32)
    for ky in range(3):
        for xa in range(10):
            for xo in range(8):
                kx = xa - xo
                if 0 <= kx <= 2:
                    for co in range(6):
                        for d in range(2):
                            wchk[ky, _rbloc(xa, d), xo * 6 + co] = cw0[co, d, ky, kx]
    # green_add = [m0, g1, m3, m0, g0, m3]; g0 = m1 - d0, g1 = m2 - d1
    for xo in range(8):
        wchk[1, _rbloc(xo + 1, 1), xo * 6 + 1] += -1.0
        wchk[1, _rbloc(xo + 1, 0), xo * 6 + 4] += -1.0
    wchm = np.zeros((128, 48), np.float32)
    for xo in range(8):
        xa = xo + 1
        wchm[_r0loc(1, 0, xa), xo * 6 + 0] = 1.0
        wchm[_r0loc(1, 0, xa), xo * 6 + 3] = 1.0
        wchm[_r0loc(1, 3, xa), xo * 6 + 2] = 1.0
        wchm[_r0loc(1, 3, xa), xo * 6 + 5] = 1.0
        wchm[_r0loc(1, 2, xa), xo * 6 + 1] = 1.0
        wchm[_r0loc(1, 1, xa), xo * 6 + 4] = 1.0
    return wchk, wchm


def assemble_output(mosaic, cp_dev, g_dev):
    B = mosaic.shape[0]
    cp_dev = np.asarray(cp_dev).astype(np.float32)
    g_dev = np.asarray(g_dev).astype(np.float32)
    cp = cp_dev.reshape(B, 8, 6, H, NW).transpose(0, 2, 3, 4, 1).reshape(B, 6, H, W)
    g = g_dev.reshape(B, 8, 2, H, NW).transpose(0, 2, 3, 4, 1).reshape(B, 2, H, W)
    m = mosaic
    out = np.empty((B, 3, 2 * H, 2 * W), np.float32)
    out[:, 0, 0::2, 0::2] = cp[:, 0]
    out[:, 0, 0::2, 1::2] = m[:, 1]
    out[:, 0, 1::2, 0::2] = cp[:, 1]
    out[:, 0, 1::2, 1::2] = cp[:, 2]
    out[:, 1, 0::2, 0::2] = m[:, 0]
    out[:, 1, 0::2, 1::2] = g[:, 0]
    out[:, 1, 1::2, 0::2] = g[:, 1]
    out[:, 1, 1::2, 1::2] = m[:, 3]
    out[:, 2, 0::2, 0::2] = cp[:, 3]
    out[:, 2, 0::2, 1::2] = cp[:, 4]
    out[:, 2, 1::2, 0::2] = m[:, 2]
    out[:, 2, 1::2, 1::2] = cp[:, 5]
    return out


# column offsets inside the packed [128, 2336] stationary tensor.
# Big stationaries are zero-padded to 128 columns: FWL (fast weight load)
# only engages when the weight has a full 128-column footprint.
_WOFF = {"wf0": 0, "ww0": 128, "wf1": 256, "wf2": 640, "ww1": 1024,
         "ww2": 1408, "wsep": 1792, "wse2": 1808, "wchk": 1824, "wchm": 2208}
_WCOLS = 2336


def pack_stationaries(st):
    wp = np.zeros((128, _WCOLS), np.float32)
    wp[:, 0:96] = st["wf0"]
    wp[:, 128:224] = st["ww0"]
    for nm in ("wf1", "wf2", "ww1", "ww2"):
        o = _WOFF[nm]
        for ky in range(3):
            wp[0:120, o + 128 * ky : o + 128 * ky + 96] = st[nm][ky]
    wp[0:96, 1792:1808] = st["wsep"]
    wp[0:96, 1808:1824] = st["wse2"]
    for ky in range(3):
        wp[0:20, 1824 + 128 * ky : 1824 + 128 * ky + 48] = st["wchk"][ky]
    wp[:, 2208:2256] = st["wchm"]
    return wp.astype(NPBF)


def build_program():
    from contextlib import ExitStack

    nc = bacc.Bacc(
        "TRN2", target_bir_lowering=False, debug=False, num_devices=N_CORES
    )
    r0 = nc.declare_dram_parameter("r0", [B_PC, 128, H, NW], BF, isOutput=False)
    wpack = nc.declare_dram_parameter("wpack", [128, _WCOLS], BF, isOutput=False)
    out_cp = nc.declare_dram_parameter("out_cp", [B_PC, 48, H, NW], BF, isOutput=True)
    out_g = nc.declare_dram_parameter("out_g", [B_PC, 16, H, NW], BF, isOutput=True)

    Relu = mybir.ActivationFunctionType.Relu
    Exp = mybir.ActivationFunctionType.Exp
    Copy = mybir.ActivationFunctionType.Copy
    Mult = mybir.AluOpType.mult
    Sub = mybir.AluOpType.subtract
    Max = mybir.AluOpType.max

    with tile.TileContext(nc) as tc, ExitStack() as ctx:
        const = ctx.enter_context(tc.tile_pool(name="const", bufs=1))
        r0pool = ctx.enter_context(tc.tile_pool(name="r0pool", bufs=8))
        p_rfw1 = ctx.enter_context(tc.tile_pool(name="rfw1", bufs=3))
        p_rfw2 = ctx.enter_context(tc.tile_pool(name="rfw2", bufs=3))
        p_grb = ctx.enter_context(tc.tile_pool(name="grb", bufs=3))
        p_stg = ctx.enter_context(tc.tile_pool(name="stg", bufs=2))
        p_d = ctx.enter_context(tc.tile_pool(name="dbuf", bufs=2))
        p_act = ctx.enter_context(tc.tile_pool(name="acts", bufs=2))
        p_eg = ctx.enter_context(tc.tile_pool(name="eg", bufs=3))
        p_sml = ctx.enter_context(tc.tile_pool(name="sml", bufs=2))
        ps_mm = ctx.enter_context(tc.tile_pool(name="psmm", bufs=5, space="PSUM"))
        ps_sm = ctx.enter_context(tc.tile_pool(name="pssm", bufs=2, space="PSUM"))
        ps_cp = ctx.enter_context(tc.tile_pool(name="pscp", bufs=1, space="PSUM"))

        WC = const.tile([128, _WCOLS], BF, tag="wpack_sb", name="wpack_sb")
        nc.sync.dma_start(out=WC[:], in_=wpack[:])
        sb = {
            "wf0": WC[:, 0:128],
            "ww0": WC[:, 128:256],
            "wsep": WC[0:96, 1792:1808],
            "wse2": WC[0:96, 1808:1824],
            "wchm": WC[:, 2208:2336],
        }

        def wky(nm, ky):
            o = _WOFF[nm]
            if nm == "wchk":
                return WC[0:20, o + 128 * ky : o + 128 * (ky + 1)]
            return WC[0:120, o + 128 * ky : o + 128 * (ky + 1)]

        gp = nc.gpsimd
        r0s, rfw1, rfw2, grb, egs = {}, {}, {}, {}, {}

        def new_rfw(pool, dct, G, gl):
            t = pool.tile([120, 2, RG + 2, NW], BF, name="t")
            dct[G] = t
            gp.memset(t[96:120, :, :, 0:1], 0.0)
            gp.memset(t[96:120, :, :, 63:64], 0.0)
            if gl == 0:
                gp.memset(t[0:96, :, 0:1, :], 0.0)
            else:
                gp.tensor_copy(out=t[0:96, :, 0:1, :], in_=dct[G - 1][0:96, :, 32:33, :])
            if gl == GPI - 1:
                gp.memset(t[0:96, :, 33:34, :], 0.0)
            return t

        def strips_main(t):
            # rows 0:33 are complete once this group's own evicts + row-0 halo
            # land; row 33 arrives later from the next group's first evict.
            # Per-tree DMAs keep the APs at 3 dims.
            for tr in (0, 1):
                gp.dma_start(
                    out=t[96:108, tr, 0:33, 1:NW],
                    in_=t[84:96, tr, 0:33, 0 : NW - 1],
                )
                gp.dma_start(
                    out=t[108:120, tr, 0:33, 0 : NW - 1],
                    in_=t[0:12, tr, 0:33, 1:NW],
                )

        def strips_last(t):
            nc.sync.dma_start(
                out=t[96:108, :, 33:34, 1:NW], in_=t[84:96, :, 33:34, 0 : NW - 1]
            )
            nc.sync.dma_start(
                out=t[108:120, :, 33:34, 0 : NW - 1], in_=t[0:12, :, 33:34, 1:NW]
            )

        def conv_int(nm, tr, t1, k):
            ps = ps_mm.tile([128, 8, NW], F32, tag="mm96", name="psc")
            for ky in range(3):
                nc.tensor.matmul(
                    ps[:],
                    wky(nm, ky),
                    t1[0:120, tr, 8 * k + ky : 8 * k + ky + 8, :],
                    start=(ky == 0),
                    stop=(ky == 2),
                )
            return ps

        for T in range(NG + 4):
            # ---- stage 0: layer-0 convs for group T
            if T < NG:
                G = T
                img, gl = divmod(G, GPI)
                if G == 0:
                    rt = r0pool.tile([128, RG, NW], BF, name="rt")
                    r0s[0] = rt
                    nc.sync.dma_start(out=rt[:], in_=r0[0, :, 0:RG, :])
                if G + 1 < NG:
                    im2, gl2 = divmod(G + 1, GPI)
                    rtn = r0pool.tile([128, RG, NW], BF, name="rtn")
                    r0s[G + 1] = rtn
                    nc.sync.dma_start(
                        out=rtn[:], in_=r0[im2, :, gl2 * RG : gl2 * RG + RG, :]
                    )
                rt = r0s[G]
                t1 = new_rfw(p_rfw1, rfw1, G, gl)
                for k in range(GROUP):
                    psf = ps_mm.tile([128, 8, NW], F32, tag="mm96", name="psf0")
                    nc.tensor.matmul(
                        psf[:], sb["wf0"], rt[:, 8 * k : 8 * k + 8, :],
                        start=True, stop=True,
                    )
                    nc.vector.tensor_scalar_max(
                        t1[0:96, 0, 8 * k + 1 : 8 * k + 9, :], psf[0:96, :, :], 0.0
                    )
                    psw = ps_mm.tile([128, 8, NW], F32, tag="mm96", name="psw0")
                    nc.tensor.matmul(
                        psw[:], sb["ww0"], rt[:, 8 * k : 8 * k + 8, :],
                        start=True, stop=True,
                    )
                    nc.scalar.activation(
                        out=t1[0:96, 1, 8 * k + 1 : 8 * k + 9, :],
                        in_=psw[0:96, :, :], func=Relu,
                    )
                    if k == 0 and gl > 0:
                        gp.tensor_copy(
                            out=rfw1[G - 1][0:96, :, 33:34, :],
                            in_=t1[0:96, :, 1:2, :],
                        )
                strips_main(t1)

            # ---- stage 1: layer-1 convs for group T-1
            if 0 <= T - 1 < NG:
                G = T - 1
                img, gl = divmod(G, GPI)
                t1 = rfw1[G]
                strips_last(t1)
                t2 = new_rfw(p_rfw2, rfw2, G, gl)
                for k in range(GROUP):
                    psf = conv_int("wf1", 0, t1, k)
                    nc.vector.tensor_scalar_max(
                        t2[0:96, 0, 8 * k + 1 : 8 * k + 9, :], psf[0:96, :, :], 0.0
                    )
                    psw = conv_int("ww1", 1, t1, k)
                    nc.scalar.activation(
                        out=t2[0:96, 1, 8 * k + 1 : 8 * k + 9, :],
                        in_=psw[0:96, :, :], func=Relu,
                    )
                    if k == 0 and gl > 0:
                        gp.tensor_copy(
                            out=rfw2[G - 1][0:96, :, 33:34, :],
                            in_=t2[0:96, :, 1:2, :],
                        )
                strips_main(t2)

            # ---- stage 2a: layer-2 convs + exp path into group tiles, T-2
            if 0 <= T - 2 < NG:
                G = T - 2
                img, gl = divmod(G, GPI)
                t2 = rfw2[G]
                strips_last(t2)
                Emg = p_eg.tile([96, GROUP, 8, NW], BF, tag="Em", name="Emg")
                EPg = p_eg.tile([96, GROUP, 8, NW], BF, tag="EP", name="EPg")
                egs[G] = (Emg, EPg)
                for k in range(GROUP):
                    psf = conv_int("wf2", 0, t2, k)
                    psw = conv_int("ww2", 1, t2, k)
                    Et = p_act.tile([96, 8, NW], BF, tag="Et", name="Et")
                    nc.scalar.activation(out=Et[:], in_=psw[0:96, :, :], func=Relu)
                    nc.scalar.activation(out=Emg[:, k, :, :], in_=Et[:], func=Exp)
                    nc.vector.scalar_tensor_tensor(
                        out=EPg[:, k, :, :], in0=psf[0:96, :, :], scalar=0.0,
                        in1=Emg[:, k, :, :], op0=Max, op1=Mult,
                    )

            # ---- stage 2b: softmax-combine into g for group T-3
            if 0 <= T - 3 < NG:
                G = T - 3
                img, gl = divmod(G, GPI)
                Emg, EPg = egs[G]
                gt = p_grb.tile([84, RG + 2, NW], BF, name="gt")
                grb[G] = gt
                gp.memset(gt[64:84, :, 0:1], 0.0)
                gp.memset(gt[64:84, :, 63:64], 0.0)
                if gl == 0:
                    gp.memset(gt[64:84, 0:1, :], 0.0)
                else:
                    gp.tensor_copy(
                        out=gt[64:80, 0:1, :], in_=grb[G - 1][64:80, 32:33, :]
                    )
                if gl == GPI - 1:
                    gp.memset(gt[64:84, 33:34, :], 0.0)
                sm = {}

                def finish(j, gt=gt, sm=sm, G=G, gl=gl):
                    pse, psep = sm.pop(j)
                    rcp = p_sml.tile([16, 8, NW], F32, tag="rcp", name="rcp")
                    nc.vector.reciprocal_approx_fast(out=rcp[:], in_=pse[:])
                    nc.vector.tensor_mul(
                        gt[64:80, 8 * j + 1 : 8 * j + 9, :], psep[:], rcp[:]
                    )
                    if j == 0 and gl > 0:
                        gp.tensor_copy(
                            out=grb[G - 1][64:80, 33:34, :], in_=gt[64:80, 1:2, :]
                        )

                for k in range(GROUP):
                    if k >= 1:
                        finish(k - 1)
                    pse = ps_sm.tile([16, 8, NW], F32, tag="sm", name="pse")
                    nc.tensor.matmul(
                        pse[:], sb["wse2"], Emg[:, k, :, :], start=True, stop=True
                    )
                    psep = ps_sm.tile([16, 8, NW], F32, tag="sm", name="psep")
                    nc.tensor.matmul(
                        psep[:], sb["wsep"], EPg[:, k, :, :], start=True, stop=True
                    )
                    sm[k] = (pse, psep)
                finish(GROUP - 1)
                nc.sync.dma_start(
                    out=gt[80:82, 0:33, 1:NW], in_=gt[78:80, 0:33, 0 : NW - 1]
                )
                nc.sync.dma_start(
                    out=gt[82:84, 0:33, 0 : NW - 1], in_=gt[64:66, 0:33, 1:NW]
                )

            # ---- stage 3: chroma + outputs for group T-4
            if 0 <= T - 4 < NG:
                G = T - 4
                img, gl = divmod(G, GPI)
                y0 = gl * RG
                gt = grb[G]
                nc.sync.dma_start(
                    out=gt[80:82, 33:34, 1:NW], in_=gt[78:80, 33:34, 0 : NW - 1]
                )
                nc.sync.dma_start(
                    out=gt[82:84, 33:34, 0 : NW - 1], in_=gt[64:66, 33:34, 1:NW]
                )
                rt = r0s[G]
                st = p_stg.tile([48, RG, NW], BF, name="st")
                for k in range(GROUP):
                    d = p_d.tile([20, 10, NW], BF, name="d")
                    lo = 1 if k == 0 else 0
                    hi = 9 if k == GROUP - 1 else 10
                    nc.vector.scalar_tensor_tensor(
                        out=d[:, lo:hi, :],
                        in0=rt[64:84, 8 * k + lo - 1 : 8 * k + hi - 1, :],
                        scalar=1.0,
                        in1=gt[64:84, 8 * k + lo : 8 * k + hi, :],
                        op0=Mult,
                        op1=Sub,
                    )
                    if k == 0:
                        if gl > 0:
                            nc.vector.scalar_tensor_tensor(
                                out=d[:, 0:1, :],
                                in0=r0s[G - 1][64:84, RG - 1 : RG, :], scalar=1.0,
                                in1=gt[64:84, 0:1, :], op0=Mult, op1=Sub,
                            )
                        else:
                            nc.vector.memset(d[:, 0:1, :], 0.0)
                    if k == GROUP - 1:
                        if gl < GPI - 1:
                            nc.vector.scalar_tensor_tensor(
                                out=d[:, 9:10, :],
                                in0=r0s[G + 1][64:84, 0:1, :], scalar=1.0,
                                in1=gt[64:84, 8 * k + 9 : 8 * k + 10, :],
                                op0=Mult, op1=Sub,
                            )
                        else:
                            nc.vector.memset(d[:, 9:10, :], 0.0)
                    pc = ps_cp.tile([128, 8, NW], F32, tag="cp", name="pc")
                    for ky in range(3):
                        nc.tensor.matmul(
                            pc[:], wky("wchk", ky), d[:, ky : ky + 8, :],
                            start=(ky == 0), stop=False,
                        )
                    nc.tensor.matmul(
                        pc[:], sb["wchm"], rt[:, 8 * k : 8 * k + 8, :],
                        start=False, stop=True,
                    )
                    nc.scalar.activation(
                        out=st[0:48, 8 * k : 8 * k + 8, :], in_=pc[0:48, :, :],
                        func=Copy,
                    )
                nc.sync.dma_start(out=out_cp[img, :, y0 : y0 + RG, :], in_=st[:])
                nc.sync.dma_start(
                    out=out_g[img, :, y0 : y0 + RG, :], in_=gt[64:80, 1 : RG + 1, :]
                )
                for dct in (r0s, rfw1, rfw2, grb, egs):
                    dct.pop(G - 2, None)

    nc.compile()
    return nc


_CACHE = {}


def kernel(mosaic, fw0, fw1, fw2, ww0, ww1, ww2, cw0, _trace=False):
    mosaic = np.asarray(mosaic, np.float32)
    r0_all = build_r0(mosaic)

    stat = {
        "wf0": build_w_l0(np.asarray(fw0, np.float32)),
        "ww0": build_w_l0(np.asarray(ww0, np.float32)),
        "wf1": build_w_int(np.asarray(fw1, np.float32)),
        "wf2": build_w_int(np.asarray(fw2, np.float32)),
        "ww1": build_w_int(np.asarray(ww1, np.float32)),
        "ww2": build_w_int(np.asarray(ww2, np.float32)),
    }
    stat["wse2"], stat["wsep"] = build_w_sums()
    stat["wchk"], stat["wchm"] = build_w_chroma(np.asarray(cw0, np.float32))
    wpack = pack_stationaries(stat)

    if "nc" not in _CACHE:
        _CACHE["nc"] = build_program()
    nc = _CACHE["nc"]

    in_maps = []
    for c in range(N_CORES):
        in_maps.append(
            {"r0": np.ascontiguousarray(r0_all[c * B_PC : (c + 1) * B_PC]),
             "wpack": wpack}
        )

    res = run_bass_kernel_spmd(nc, in_maps, list(range(N_CORES)), trace=_trace)
    outs = []
    for c in range(N_CORES):
        outs.append(
            assemble_output(
                mosaic[c * B_PC : (c + 1) * B_PC],
                res.results[c]["out_cp"],
                res.results[c]["out_g"],
            )
        )
    full = np.concatenate(outs, axis=0)
    if _trace:
        return full, res
    return full


# revision 16
# speedup vs baseline: 1.0142x; 1.0142x over previous
"""Trainium2 Bass kernel for BasicQuadRGBModel (quad-Bayer demosaic CNN).

Data-parallel over 8 cores (2 images each). bf16 datapath, f32 PSUM.

Design (v7):
  - conv = 3 accumulating matmuls/layer (K=120, M=96 padded to 128, N=512 =
    8 rows x 64 win). Stationaries zero-padded to 128 columns so the PE's
    fast-weight-load path (FWL, 128-col requirement) engages.
  - Row-slab steps of 8 rows grouped 4-at-a-time into [*, 34, 64] SBUF tiles
    (32 rows + 1 halo row each side): halo copies, edge memsets, strip DMAs
    and HBM in/out DMAs amortize 4x. f/w tree pairs share one
    [120, 2, 34, 64] tile per layer so one strip DMA serves both trees.
  - Strip DMAs split rows 0:33 (issued by the producing stage) from row 33
    (tiny DMA in the consuming stage) so layer N+1's first convs never wait
    on the next group's halo; r0 input is prefetched one group ahead.
  - 5-deep software pipeline (L0 / L1 / L2+exp / softmax-combine / chroma);
    the softmax-combine matmuls run one group behind the exp stage so the
    PE in-order stream never waits on Act/DVE results of the same step.
  - Softmax: E = exp(relu(w)) via Act relu+exp; EP = relu(f)*E fused on DVE;
    the sum matmul duplicates both half-slots (M=16) so
    reciprocal_approx_fast applies directly (no broadcast matmul); g = EP
    sums * 1/E sums on DVE.
  - g lives at partition base 64 so d = rb - g is a single fused DVE op
    against rt[64:84] (equal-SBUF-base rule for 2-input ops).
  - Engine placement: Act = L0w/L1w evictions + relu/exp + chroma staging;
    DVE = L0f/L1f evictions + fused softmax/chroma ops; GpSimd = halo
    copies, memsets, bulk strip DMA issue; Sync = HBM in/out + small strips.
  - Host does layer-0 im2col (r0) and the final 2x2 pixel shuffle.
"""

import sys

sys.path.insert(0, "/opt/trn_rl_repo")

import ml_dtypes
import numpy as np

import concourse.bass as bass
import concourse.mybir as mybir
import concourse.tile as tile
from concourse import bacc
from concourse.bass_utils import run_bass_kernel_spmd

N_CORES = 8
B_PC = 2
H = W = 512
NW = 64
NSLAB = 64
GROUP = 4
RG = 8 * GROUP  # rows per group
GPI = NSLAB // GROUP  # groups per image
NG = B_PC * GPI
CH = 12
F32 = mybir.dt.float32
BF = mybir.dt.bfloat16
NPBF = ml_dtypes.bfloat16


def _rbloc(xa, c):
    if xa == 0:
        return 16 + c
    if xa == 9:
        return 18 + c
    return (xa - 1) * 2 + c


def _rloc(xa, ci):
    if xa == 0:
        return 96 + ci
    if xa == 9:
        return 108 + ci
    return (xa - 1) * 12 + ci


def _r0loc(ky, ci, xa):
    if ky == 0:
        if ci == 0:
            return xa
        if ci == 3:
            return 10 + xa
        return 20 + _rbloc(xa, ci - 1)
    if ky == 1:
        if ci == 0:
            return 40 + xa
        if ci == 3:
            return 50 + xa
        return 64 + _rbloc(xa, ci - 1)
    if ci == 0:
        return 84 + xa
    if ci == 3:
        return 94 + xa
    return 104 + _rbloc(xa, ci - 1)


def build_r0(mosaic):
    B = mosaic.shape[0]
    mp = np.zeros((B, 4, H + 2, W + 2), np.float32)
    mp[:, :, 1 : H + 1, 1 : W + 1] = mosaic
    r0 = np.zeros((B, 128, H, NW), np.float32)
    for ky in range(3):
        for ci in range(4):
            for xa in range(10):
                r0[:, _r0loc(ky, ci, xa)] = mp[:, ci, ky : ky + H, xa : xa + 8 * NW : 8]
    return r0.astype(NPBF)


def build_w_l0(wt):
    W_ = np.zeros((128, 96), np.float32)
    for ky in range(3):
        for ci in range(4):
            for xa in range(10):
                for xo in range(8):
                    kx = xa - xo
                    if 0 <= kx <= 2:
                        for co in range(CH):
                            W_[_r0loc(ky, ci, xa), xo * 12 + co] = wt[co, ci, ky, kx]
    return W_


def build_w_int(wt):
    W_ = np.zeros((3, 120, 96), np.float32)
    for ky in range(3):
        for xa in range(10):
            for xo in range(8):
                kx = xa - xo
                if 0 <= kx <= 2:
                    k = _rloc(xa, 0)
                    W_[ky, k : k + 12, xo * 12 : xo * 12 + 12] = wt[:, :, ky, kx].T
    return W_


def build_w_sums():
    # wse2 duplicates the full per-xo sum into both half slots so the
    # reciprocal applies directly to the 16-partition layout (no broadcast
    # matmul needed).
    wse2 = np.zeros((96, 16), np.float32)
    wsep = np.zeros((96, 16), np.float32)
    for xo in range(8):
        for co in range(CH):
            wse2[xo * 12 + co, xo * 2] = 1.0
            wse2[xo * 12 + co, xo * 2 + 1] = 1.0
            wsep[xo * 12 + co, xo * 2 + (co >= 6)] = 1.0
    return wse2, wsep


def build_w_chroma(cw0):
    wchk = np.zeros((3, 20, 48), np.float32)
    for ky in range(3):
        for xa in range(10):
            for xo in range(8):
                kx = xa - xo
                if 0 <= kx <= 2:
                    for co in range(6):
                        for d in range(2):
                            wchk[ky, _rbloc(xa, d), xo * 6 + co] = cw0[co, d, ky, kx]
    # green_add = [m0, g1, m3, m0, g0, m3]; g0 = m1 - d0, g1 = m2 - d1
    for xo in range(8):
        wchk[1, _rbloc(xo + 1, 1), xo * 6 + 1] += -1.0
        wchk[1, _rbloc(xo + 1, 0), xo * 6 + 4] += -1.0
    wchm = np.zeros((128, 48), np.float32)
    for xo in range(8):
        xa = xo + 1
        wchm[_r0loc(1, 0, xa), xo * 6 + 0] = 1.0
        wchm[_r0loc(1, 0, xa), xo * 6 + 3] = 1.0
        wchm[_r0loc(1, 3, xa), xo * 6 + 2] = 1.0
        wchm[_r0loc(1, 3, xa), xo * 6 + 5] = 1.0
        wchm[_r0loc(1, 2, xa), xo * 6 + 1] = 1.0
        wchm[_r0loc(1, 1, xa), xo * 6 + 4] = 1.0
    return wchk, wchm


def assemble_output(mosaic, cp_dev, g_dev):
    B = mosaic.shape[0]
    cp_dev = np.asarray(cp_dev).astype(np.float32)
    g_dev = np.asarray(g_dev).astype(np.float32)
    cp = cp_dev.reshape(B, 8, 6, H, NW).transpose(0, 2, 3, 4, 1).reshape(B, 6, H, W)
    g = g_dev.reshape(B, 8, 2, H, NW).transpose(0, 2, 3, 4, 1).reshape(B, 2, H, W)
    m = mosaic
    out = np.empty((B, 3, 2 * H, 2 * W), np.float32)
    out[:, 0, 0::2, 0::2] = cp[:, 0]
    out[:, 0, 0::2, 1::2] = m[:, 1]
    out[:, 0, 1::2, 0::2] = cp[:, 1]
    out[:, 0, 1::2, 1::2] = cp[:, 2]
    out[:, 1, 0::2, 0::2] = m[:, 0]
    out[:, 1, 0::2, 1::2] = g[:, 0]
    out[:, 1, 1::2, 0::2] = g[:, 1]
    out[:, 1, 1::2, 1::2] = m[:, 3]
    out[:, 2, 0::2, 0::2] = cp[:, 3]
    out[:, 2, 0::2, 1::2] = cp[:, 4]
    out[:, 2, 1::2, 0::2] = m[:, 2]
    out[:, 2, 1::2, 1::2] = cp[:, 5]
    return out


# column offsets inside the packed [128, 2336] stationary tensor.
# Big stationaries are zero-padded to 128 columns: FWL (fast weight load)
# only engages when the weight has a full 128-column footprint.
_WOFF = {"wf0": 0, "ww0": 128, "wf1": 256, "wf2": 640, "ww1": 1024,
         "ww2": 1408, "wsep": 1792, "wse2": 1808, "wchk": 1824, "wchm": 2208}
_WCOLS = 2336


def pack_stationaries(st):
    wp = np.zeros((128, _WCOLS), np.float32)
    wp[:, 0:96] = st["wf0"]
    wp[:, 128:224] = st["ww0"]
    for nm in ("wf1", "wf2", "ww1", "ww2"):
        o = _WOFF[nm]
        for ky in range(3):
            wp[0:120, o + 128 * ky : o + 128 * ky + 96] = st[nm][ky]
    wp[0:96, 1792:1808] = st["wsep"]
    wp[0:96, 1808:1824] = st["wse2"]
    for ky in range(3):
        wp[0:20, 1824 + 128 * ky : 1824 + 128 * ky + 48] = st["wchk"][ky]
    wp[:, 2208:2256] = st["wchm"]
    return wp.astype(NPBF)


def build_program():
    from contextlib import ExitStack

    nc = bacc.Bacc(
        "TRN2", target_bir_lowering=False, debug=False, num_devices=N_CORES
    )
    r0 = nc.declare_dram_parameter("r0", [B_PC, 128, H, NW], BF, isOutput=False)
    wpack = nc.declare_dram_parameter("wpack", [128, _WCOLS], BF, isOutput=False)
    out_cp = nc.declare_dram_parameter("out_cp", [B_PC, 48, H, NW], BF, isOutput=True)
    out_g = nc.declare_dram_parameter("out_g", [B_PC, 16, H, NW], BF, isOutput=True)

    Relu = mybir.ActivationFunctionType.Relu
    Exp = mybir.ActivationFunctionType.Exp
    Copy = mybir.ActivationFunctionType.Copy
    Mult = mybir.AluOpType.mult
    Sub = mybir.AluOpType.subtract
    Max = mybir.AluOpType.max

    with tile.TileContext(nc) as tc, ExitStack() as ctx:
        const = ctx.enter_context(tc.tile_pool(name="const", bufs=1))
        r0pool = ctx.enter_context(tc.tile_pool(name="r0pool", bufs=8))
        p_rfw1 = ctx.enter_context(tc.tile_pool(name="rfw1", bufs=4))
        p_rfw2 = ctx.enter_context(tc.tile_pool(name="rfw2", bufs=4))
        p_grb = ctx.enter_context(tc.tile_pool(name="grb", bufs=3))
        p_stg = ctx.enter_context(tc.tile_pool(name="stg", bufs=3))
        p_d = ctx.enter_context(tc.tile_pool(name="dbuf", bufs=3))
        p_act = ctx.enter_context(tc.tile_pool(name="acts", bufs=3))
        p_eg = ctx.enter_context(tc.tile_pool(name="eg", bufs=3))
        p_sml = ctx.enter_context(tc.tile_pool(name="sml", bufs=3))
        ps_mm = ctx.enter_context(tc.tile_pool(name="psmm", bufs=5, space="PSUM"))
        ps_sm = ctx.enter_context(tc.tile_pool(name="pssm", bufs=2, space="PSUM"))
        ps_cp = ctx.enter_context(tc.tile_pool(name="pscp", bufs=1, space="PSUM"))

        WC = const.tile([128, _WCOLS], BF, tag="wpack_sb", name="wpack_sb")
        nc.sync.dma_start(out=WC[:], in_=wpack[:])
        sb = {
            "wf0": WC[:, 0:128],
            "ww0": WC[:, 128:256],
            "wsep": WC[0:96, 1792:1808],
            "wse2": WC[0:96, 1808:1824],
            "wchm": WC[:, 2208:2336],
        }

        def wky(nm, ky):
            o = _WOFF[nm]
            if nm == "wchk":
                return WC[0:20, o + 128 * ky : o + 128 * (ky + 1)]
            return WC[0:120, o + 128 * ky : o + 128 * (ky + 1)]

        gp = nc.gpsimd
        r0s, rfw1, rfw2, grb, egs = {}, {}, {}, {}, {}

        def new_rfw(pool, dct, G, gl):
            t = pool.tile([120, 2, RG + 2, NW], BF, name="t")
            dct[G] = t
            gp.memset(t[96:120, :, :, 0:1], 0.0)
            gp.memset(t[96:120, :, :, 63:64], 0.0)
            if gl == 0:
                gp.memset(t[0:96, :, 0:1, :], 0.0)
            else:
                gp.tensor_copy(out=t[0:96, :, 0:1, :], in_=dct[G - 1][0:96, :, 32:33, :])
            if gl == GPI - 1:
                gp.memset(t[0:96, :, 33:34, :], 0.0)
            return t

        def strips_main(t):
            # rows 0:33 are complete once this group's own evicts + row-0 halo
            # land; row 33 arrives later from the next group's first evict.
            # Per-tree DMAs keep the APs at 3 dims.
            for tr in (0, 1):
                gp.dma_start(
                    out=t[96:108, tr, 0:33, 1:NW],
                    in_=t[84:96, tr, 0:33, 0 : NW - 1],
                )
                gp.dma_start(
                    out=t[108:120, tr, 0:33, 0 : NW - 1],
                    in_=t[0:12, tr, 0:33, 1:NW],
                )

        def strips_last(t):
            nc.sync.dma_start(
                out=t[96:108, :, 33:34, 1:NW], in_=t[84:96, :, 33:34, 0 : NW - 1]
            )
            nc.sync.dma_start(
                out=t[108:120, :, 33:34, 0 : NW - 1], in_=t[0:12, :, 33:34, 1:NW]
            )

        def conv_int(nm, tr, t1, k):
            ps = ps_mm.tile([128, 8, NW], F32, tag="mm96", name="psc")
            for ky in range(3):
                nc.tensor.matmul(
                    ps[:],
                    wky(nm, ky),
                    t1[0:120, tr, 8 * k + ky : 8 * k + ky + 8, :],
                    start=(ky == 0),
                    stop=(ky == 2),
                )
            return ps

        for T in range(NG + 4):
            # ---- stage 0: layer-0 convs for group T
            if T < NG:
                G = T
                img, gl = divmod(G, GPI)
                if G == 0:
                    rt = r0pool.tile([128, RG, NW], BF, name="rt")
                    r0s[0] = rt
                    nc.sync.dma_start(out=rt[:], in_=r0[0, :, 0:RG, :])
                if G + 1 < NG:
                    im2, gl2 = divmod(G + 1, GPI)
                    rtn = r0pool.tile([128, RG, NW], BF, name="rtn")
                    r0s[G + 1] = rtn
                    nc.sync.dma_start(
                        out=rtn[:], in_=r0[im2, :, gl2 * RG : gl2 * RG + RG, :]
                    )
                rt = r0s[G]
                t1 = new_rfw(p_rfw1, rfw1, G, gl)
                for k in range(GROUP):
                    psf = ps_mm.tile([128, 8, NW], F32, tag="mm96", name="psf0")
                    nc.tensor.matmul(
                        psf[:], sb["wf0"], rt[:, 8 * k : 8 * k + 8, :],
                        start=True, stop=True,
                    )
                    nc.vector.tensor_scalar_max(
                        t1[0:96, 0, 8 * k + 1 : 8 * k + 9, :], psf[0:96, :, :], 0.0
                    )
                    psw = ps_mm.tile([128, 8, NW], F32, tag="mm96", name="psw0")
                    nc.tensor.matmul(
                        psw[:], sb["ww0"], rt[:, 8 * k : 8 * k + 8, :],
                        start=True, stop=True,
                    )
                    nc.scalar.activation(
                        out=t1[0:96, 1, 8 * k + 1 : 8 * k + 9, :],
                        in_=psw[0:96, :, :], func=Relu,
                    )
                    if k == 0 and gl > 0:
                        gp.tensor_copy(
                            out=rfw1[G - 1][0:96, :, 33:34, :],
                            in_=t1[0:96, :, 1:2, :],
                        )
                strips_main(t1)

            # ---- stage 1: layer-1 convs for group T-1
            if 0 <= T - 1 < NG:
                G = T - 1
                img, gl = divmod(G, GPI)
                t1 = rfw1[G]
                strips_last(t1)
                t2 = new_rfw(p_rfw2, rfw2, G, gl)
                for k in range(GROUP):
                    psf = conv_int("wf1", 0, t1, k)
                    nc.vector.tensor_scalar_max(
                        t2[0:96, 0, 8 * k + 1 : 8 * k + 9, :], psf[0:96, :, :], 0.0
                    )
                    psw = conv_int("ww1", 1, t1, k)
                    nc.scalar.activation(
                        out=t2[0:96, 1, 8 * k + 1 : 8 * k + 9, :],
                        in_=psw[0:96, :, :], func=Relu,
                    )
                    if k == 0 and gl > 0:
                        gp.tensor_copy(
                            out=rfw2[G - 1][0:96, :, 33:34, :],
                            in_=t2[0:96, :, 1:2, :],
                        )
                strips_main(t2)

            # ---- stage 2a: layer-2 convs + exp path into group tiles, T-2
            if 0 <= T - 2 < NG:
                G = T - 2
                img, gl = divmod(G, GPI)
                t2 = rfw2[G]
                strips_last(t2)
                Emg = p_eg.tile([96, GROUP, 8, NW], BF, tag="Em", name="Emg")
                EPg = p_eg.tile([96, GROUP, 8, NW], BF, tag="EP", name="EPg")
                egs[G] = (Emg, EPg)
                for k in range(GROUP):
                    psf = conv_int("wf2", 0, t2, k)
                    psw = conv_int("ww2", 1, t2, k)
                    Et = p_act.tile([96, 8, NW], BF, tag="Et", name="Et")
                    nc.scalar.activation(out=Et[:], in_=psw[0:96, :, :], func=Relu)
                    nc.scalar.activation(out=Emg[:, k, :, :], in_=Et[:], func=Exp)
                    nc.vector.scalar_tensor_tensor(
                        out=EPg[:, k, :, :], in0=psf[0:96, :, :], scalar=0.0,
                        in1=Emg[:, k, :, :], op0=Max, op1=Mult,
                    )

            # ---- stage 2b: softmax-combine into g for group T-3
            if 0 <= T - 3 < NG:
                G = T - 3
                img, gl = divmod(G, GPI)
                Emg, EPg = egs[G]
                gt = p_grb.tile([84, RG + 2, NW], BF, name="gt")
                grb[G] = gt
                gp.memset(gt[64:84, :, 0:1], 0.0)
                gp.memset(gt[64:84, :, 63:64], 0.0)
                if gl == 0:
                    gp.memset(gt[64:84, 0:1, :], 0.0)
                else:
                    gp.tensor_copy(
                        out=gt[64:80, 0:1, :], in_=grb[G - 1][64:80, 32:33, :]
                    )
                if gl == GPI - 1:
                    gp.memset(gt[64:84, 33:34, :], 0.0)
                sm = {}

                def finish(j, gt=gt, sm=sm, G=G, gl=gl):
                    pse, psep = sm.pop(j)
                    rcp = p_sml.tile([16, 8, NW], F32, tag="rcp", name="rcp")
                    nc.vector.reciprocal_approx_fast(out=rcp[:], in_=pse[:])
                    nc.vector.tensor_mul(
                        gt[64:80, 8 * j + 1 : 8 * j + 9, :], psep[:], rcp[:]
                    )
                    if j == 0 and gl > 0:
                        gp.tensor_copy(
                            out=grb[G - 1][64:80, 33:34, :], in_=gt[64:80, 1:2, :]
                        )

                for k in range(GROUP):
                    if k >= 1:
                        finish(k - 1)
                    pse = ps_sm.tile([16, 8, NW], F32, tag="sm", name="pse")
                    nc.tensor.matmul(
                        pse[:], sb["wse2"], Emg[:, k, :, :], start=True, stop=True
                    )
                    psep = ps_sm.tile([16, 8, NW], F32, tag="sm", name="psep")
                    nc.tensor.matmul(
                        psep[:], sb["wsep"], EPg[:, k, :, :], start=True, stop=True
                    )
                    sm[k] = (pse, psep)
                finish(GROUP - 1)
                nc.sync.dma_start(
                    out=gt[80:82, 0:33, 1:NW], in_=gt[78:80, 0:33, 0 : NW - 1]
                )
                nc.sync.dma_start(
                    out=gt[82:84, 0:33, 0 : NW - 1], in_=gt[64:66, 0:33, 1:NW]
                )

            # ---- stage 3: chroma + outputs for group T-4
            if 0 <= T - 4 < NG:
                G = T - 4
                img, gl = divmod(G, GPI)
                y0 = gl * RG
                gt = grb[G]
                nc.sync.dma_start(
                    out=gt[80:82, 33:34, 1:NW], in_=gt[78:80, 33:34, 0 : NW - 1]
                )
                nc.sync.dma_start(
                    out=gt[82:84, 33:34, 0 : NW - 1], in_=gt[64:66, 33:34, 1:NW]
                )
                rt = r0s[G]
                st = p_stg.tile([48, RG, NW], BF, name="st")
                for k in range(GROUP):
                    d = p_d.tile([20, 10, NW], BF, name="d")
                    lo = 1 if k == 0 else 0
                    hi = 9 if k == GROUP - 1 else 10
                    nc.vector.scalar_tensor_tensor(
                        out=d[:, lo:hi, :],
                        in0=rt[64:84, 8 * k + lo - 1 : 8 * k + hi - 1, :],
                        scalar=1.0,
                        in1=gt[64:84, 8 * k + lo : 8 * k + hi, :],
                        op0=Mult,
                        op1=Sub,
                    )
                    if k == 0:
                        if gl > 0:
                            nc.vector.scalar_tensor_tensor(
                                out=d[:, 0:1, :],
                                in0=r0s[G - 1][64:84, RG - 1 : RG, :], scalar=1.0,
                                in1=gt[64:84, 0:1, :], op0=Mult, op1=Sub,
                            )
                        else:
                            nc.vector.memset(d[:, 0:1, :], 0.0)
                    if k == GROUP - 1:
                        if gl < GPI - 1:
                            nc.vector.scalar_tensor_tensor(
                                out=d[:, 9:10, :],
                                in0=r0s[G + 1][64:84, 0:1, :], scalar=1.0,
                                in1=gt[64:84, 8 * k + 9 : 8 * k + 10, :],
                                op0=Mult, op1=Sub,
                            )
                        else:
                            nc.vector.memset(d[:, 9:10, :], 0.0)
                    pc = ps_cp.tile([128, 8, NW], F32, tag="cp", name="pc")
                    for ky in range(3):
                        nc.tensor.matmul(
                            pc[:], wky("wchk", ky), d[:, ky : ky + 8, :],
                            start=(ky == 0), stop=False,
                        )
                    nc.tensor.matmul(
                        pc[:], sb["wchm"], rt[:, 8 * k : 8 * k + 8, :],
                        start=False, stop=True,
                    )
                    nc.scalar.activation(
                        out=st[0:48, 8 * k : 8 * k + 8, :], in_=pc[0:48, :, :],
                        func=Copy,
                    )
                nc.sync.dma_start(out=out_cp[img, :, y0 : y0 + RG, :], in_=st[:])
                nc.sync.dma_start(
                    out=out_g[img, :, y0 : y0 + RG, :], in_=gt[64:80, 1 : RG + 1, :]
                )
                for dct in (r0s, rfw1, rfw2, grb, egs):
                    dct.pop(G - 2, None)

    nc.compile()
    return nc


_CACHE = {}


def kernel(mosaic, fw0, fw1, fw2, ww0, ww1, ww2, cw0, _trace=False):
    mosaic = np.asarray(mosaic, np.float32)
    r0_all = build_r0(mosaic)

    stat = {
        "wf0": build_w_l0(np.asarray(fw0, np.float32)),
        "ww0": build_w_l0(np.asarray(ww0, np.float32)),
        "wf1": build_w_int(np.asarray(fw1, np.float32)),
        "wf2": build_w_int(np.asarray(fw2, np.float32)),
        "ww1": build_w_int(np.asarray(ww1, np.float32)),
        "ww2": build_w_int(np.asarray(ww2, np.float32)),
    }
    stat["wse2"], stat["wsep"] = build_w_sums()
    stat["wchk"], stat["wchm"] = build_w_chroma(np.asarray(cw0, np.float32))
    wpack = pack_stationaries(stat)

    if "nc" not in _CACHE:
        _CACHE["nc"] = build_program()
    nc = _CACHE["nc"]

    in_maps = []
    for c in range(N_CORES):
        in_maps.append(
            {"r0": np.ascontiguousarray(r0_all[c * B_PC : (c + 1) * B_PC]),
             "wpack": wpack}
        )

    res = run_bass_kernel_spmd(nc, in_maps, list(range(N_CORES)), trace=_trace)
    outs = []
    for c in range(N_CORES):
        outs.append(
            assemble_output(
                mosaic[c * B_PC : (c + 1) * B_PC],
                res.results[c]["out_cp"],
                res.results[c]["out_g"],
            )
        )
    full = np.concatenate(outs, axis=0)
    if _trace:
        return full, res
    return full
